# revision 12
# baseline (speedup 1.0000x reference)
"""AttnBlock (GroupNorm + 1x1-conv QKV + single-head spatial attention + proj
+ residual) on 8 Trainium2 NeuronCores.

Sharding: pure data-parallel over batch — 16 samples / 8 cores = 2 samples per
core; weights broadcast. No collectives; gather on host.

Per-core formulation (per sample, C=512 channels, N=1024 spatial), all big
GEMMs in bf16 (fp32 PSUM accumulation; rel-err ~3e-3 vs fp32 reference):
  h   = groupnorm(x)                   (stats via PE indicator matmuls; h bf16)
  vT  = h^T @ v_w^T                    (spatial on partitions, C free)
  t   = M2 @ h + r                     (M2 = Wk^T Wq host-precomputed;
                                        r = Wk^T q_b folds the q-bias row term)
  s   = h^T t                          (== k^T q up to softmax-invariant terms)
  e   = exp(s * C^-0.5)                (logits O(1); no max-subtraction)
  S   = ones^T e                       (softmax denominators via PE reduction)
  o   = vT^T e * (1/S)                 (1/S broadcast across partitions via DRAM)
  y   = x + proj_w @ o + pb2           (pb2 = proj_b + proj_w @ v_b folds v-bias)
The M2 trick removes one C x C x N GEMM per sample and the q/k bias passes;
dropped scores terms are constant over keys and cancel in softmax — exact.
bf16 stationary operands keep LDWEIGHTS (~107ns) hidden under the 213ns
matmul stream, vs fp32r's 224ns two-pass load that gated the fp32 baseline.
A burst of dummy matmuls during the DMA/GroupNorm head keeps the PE HAM
un-throttled (K=8/8, 2.4GHz) when the real GEMMs arrive.
"""

import numpy as np
import ml_dtypes

import concourse.bass as bass
import concourse.tile as tile
from concourse import bacc, mybir
from concourse.bass_utils import run_bass_kernel_spmd

B, C, H, W = 16, 512, 32, 32
N = H * W              # 1024 spatial positions
G = 32                 # groups
GS = C // G            # 16 channels per group
NCORES = 8
SPC = B // NCORES      # samples per core
EPS = 1e-6
SCALE = float(C) ** -0.5
KT = C // 128          # 4 channel tiles of 128
NT = N // 128          # 8 spatial tiles of 128
NH = N // 512          # 2 free-dim halves of 512

F32 = mybir.dt.float32
F32R = mybir.dt.float32r
BF16 = mybir.dt.bfloat16
NPBF = ml_dtypes.bfloat16

WARMUP_MM = 24         # dummy PE matmuls during the head to keep HAM warm
WARMUP_MM2 = 6         # second warmup burst while the h-applies run

_BUILD_CACHE = {}
LAST_RESULT = None  # BassKernelResults of the most recent run (for test harness)


def _build():
    nc = bacc.Bacc("TRN2", target_bir_lowering=False, debug=False)

    x_ext = nc.declare_dram_parameter("x", [SPC, C, N], F32, isOutput=False)
    xbf_ext = nc.declare_dram_parameter("xbf", [SPC, C, N], BF16, isOutput=False)
    m2T_ext = nc.declare_dram_parameter("m2T", [C, C], BF16, isOutput=False)
    wvT_ext = nc.declare_dram_parameter("wvT", [C, C], BF16, isOutput=False)
    projwT_ext = nc.declare_dram_parameter("projwT", [C, C], BF16, isOutput=False)
    cst_ext = nc.declare_dram_parameter("consts12", [128, 12], F32, isOutput=False)
    rcol_ext = nc.declare_dram_parameter("r_col", [128, KT], F32, isOutput=False)
    ind_ext = nc.declare_dram_parameter("ind16", [128, 8], F32R, isOutput=False)
    indT_ext = nc.declare_dram_parameter("ind16T", [8, 128], F32R, isOutput=False)
    ones_ext = nc.declare_dram_parameter("onesb", [128], BF16, isOutput=False)
    y_ext = nc.declare_dram_parameter("y", [SPC, C, N], F32, isOutput=True)

    sdram = nc.dram_tensor("rs_bounce", [SPC, N], F32)

    Identity = mybir.ActivationFunctionType.Identity
    Exp = mybir.ActivationFunctionType.Exp
    Sqrt = mybir.ActivationFunctionType.Sqrt
    Square = mybir.ActivationFunctionType.Square
    mult = mybir.AluOpType.mult
    add = mybir.AluOpType.add

    with tile.TileContext(nc) as tc:
        with (
            tc.tile_pool(name="wpool", bufs=1) as wpool,
            tc.tile_pool(name="cpool", bufs=1) as cpool,
            tc.tile_pool(name="xpool", bufs=2) as xpool,
            tc.tile_pool(name="hpool", bufs=2) as hpool,
            tc.tile_pool(name="tpool", bufs=2) as tpool,
            tc.tile_pool(name="vpool", bufs=2) as vpool,
            tc.tile_pool(name="epool", bufs=2) as epool,
            tc.tile_pool(name="opool", bufs=2) as opool,
            tc.tile_pool(name="gnpool", bufs=2) as gnpool,
            tc.tile_pool(name="spool", bufs=2) as spool,
            tc.tile_pool(name="ps", bufs=8, space="PSUM") as ps,
        ):
            # ---- x sample 0: one DMA per kt, spread across idle queues so
            # the issue slots don't serialize and kt0 lands earliest ----
            x_tiles = []
            for s in range(SPC):
                x_tiles.append(
                    xpool.tile([128, KT, N], F32, tag="x", name=f"x_sb{s}")
                )
            # bf16 shadow of x feeds GroupNorm stats + apply: half the HBM
            # bytes of the fp32 x, so the head is off the DMA floor. The fp32
            # x (residual-only) streams in later during the quiet window.
            xbf_tiles = []
            for s in range(SPC):
                xbf_tiles.append(
                    xpool.tile([128, KT, N], BF16, tag="xbf", name=f"xbf_sb{s}")
                )
            # sg0 halves on the sync ring, sg1 halves on the gpsimd ring:
            # two rings stream in parallel and kt_i is complete early.
            for kt in range(KT):
                nc.sync.dma_start(
                    out=xbf_tiles[0][:, kt, 0:512],
                    in_=xbf_ext.ap()[0, kt * 128 : (kt + 1) * 128, 0:512],
                )
            for kt in range(KT):
                nc.gpsimd.dma_start(
                    out=xbf_tiles[0][:, kt, 512:1024],
                    in_=xbf_ext.ap()[0, kt * 128 : (kt + 1) * 128, 512:1024],
                )

            # ---- small constants (sync ring, right behind x0-sg0) ----
            cst_sb = cpool.tile([128, 12], F32)
            nc.sync.dma_start(out=cst_sb, in_=cst_ext.ap())
            nw_sb = cst_sb[:, 0:4]
            nb_sb = cst_sb[:, 4:8]
            pb_col = cst_sb[:, 8:12]
            ind_sb = cpool.tile([128, 8], F32R)
            nc.sync.dma_start(out=ind_sb, in_=ind_ext.ap())
            indT_sb = cpool.tile([8, 128], F32R)
            nc.sync.dma_start(out=indT_sb, in_=indT_ext.ap())
            eps_sb = cpool.tile([8, 1], F32)
            nc.vector.memset(eps_sb, EPS)
            # ACT table warmup: Sqrt-set then Exp-set (Identity/Square are in
            # every set, so exactly these two table loads happen)
            warm_sb = cpool.tile([8, 1], F32)
            nc.scalar.activation(out=warm_sb, in_=eps_sb, func=Sqrt)
            nc.scalar.activation(out=warm_sb, in_=eps_sb, func=Exp)

            # ---- PE warmup: keep HAM at K=8/8 through the head ----
            wmA = cpool.tile([128, 128], BF16, name="wmA")
            wmB = cpool.tile([128, 512], BF16, name="wmB")
            nc.vector.memset(wmA, 0.5)
            nc.vector.memset(wmB, 0.5)

            def pe_warm(n, base):
                for i in range(n):
                    pwt = ps.tile([128, 512], F32, tag="mm", name=f"warm{base+i}")
                    nc.tensor.matmul(pwt, wmA, wmB, start=True, stop=True)

            pe_warm(WARMUP_MM, 0)

            # ---- weights: wvT behind x0-sg1 on gpsimd (first big consumer);
            # m2T + the rest behind the consts on sync ----
            wvT_sb = wpool.tile([128, KT, C], BF16, name="wvT")
            m2T_sb = wpool.tile([128, KT, C], BF16, name="m2T")
            projw_sb = wpool.tile([128, KT, C], BF16, name="projw")
            nc.gpsimd.dma_start(
                out=wvT_sb, in_=wvT_ext.ap().rearrange("(k p) c -> p k c", p=128)
            )
            # sync ring, strictly ordered: xbf1 streams BEFORE the fat fp32
            # x loads, so sample 1's stats aren't starved by the residual data
            nc.sync.dma_start(
                out=xbf_tiles[1],
                in_=xbf_ext.ap()[1].rearrange("(k p) n -> p k n", p=128),
            )
            for s in range(SPC):
                nc.sync.dma_start(
                    out=x_tiles[s],
                    in_=x_ext.ap()[s].rearrange("(k p) n -> p k n", p=128),
                )
            nc.sync.dma_start(
                out=projw_sb,
                in_=projwT_ext.ap().rearrange("(k p) c -> p k c", p=128),
            )
            rcol_sb = cpool.tile([128, KT], F32)
            nc.sync.dma_start(out=rcol_sb, in_=rcol_ext.ap())
            ones_col = cpool.tile([128, 1], BF16)
            nc.sync.dma_start(out=ones_col, in_=ones_ext.ap().unsqueeze(1))
            nc.sync.dma_start(
                out=m2T_sb, in_=m2T_ext.ap().rearrange("(k p) c -> p k c", p=128)
            )

            def gn_stats(s, act_kts=()):
                """GroupNorm stats for sample s -> mr (8,KT,2) F32R [mean,rstd].

                Batched small-op chain: one indicator matmul for all kt.
                act_kts: kt indices whose moments run on ScalarE (Identity /
                Square with accum_out) so DVE and ACT work concurrently.
                """
                x_sb = xbf_tiles[s]
                s2_all = gnpool.tile([128, KT, 2], F32R, tag="s2", name=f"s2_{s}")
                tmp_all = gnpool.tile([128, KT], F32, tag="s2t", name=f"s2t{s}")
                for kt in range(KT):
                    if kt in act_kts:
                        s2f = gnpool.tile(
                            [128, 2], F32, tag=f"s2f{kt}", name=f"s2f{s}_{kt}"
                        )
                        scr = gnpool.tile(
                            [128, N], F32, tag="gnscr", name=f"scr{s}_{kt}"
                        )
                        nc.scalar.activation(
                            out=scr, in_=x_sb[:, kt, :], func=Identity,
                            scale=1.0 / N, accum_out=s2f[:, 0:1],
                        )
                        scr2 = gnpool.tile(
                            [128, N], F32, tag="gnscr", name=f"scr2{s}_{kt}"
                        )
                        nc.scalar.activation(
                            out=scr2, in_=x_sb[:, kt, :], func=Square,
                            scale=N ** -0.5, accum_out=s2f[:, 1:2],
                        )
                        nc.vector.tensor_copy(s2_all[:, kt, :], s2f)
                    else:
                        stats = gnpool.tile(
                            [128, 2, 6], F32, tag=f"stats{kt}", name=f"stats{s}_{kt}"
                        )
                        for sg in range(2):
                            nc.vector.bn_stats(
                                out=stats[:, sg, :],
                                in_=x_sb[:, kt, sg * 512 : (sg + 1) * 512],
                            )
                        mv = gnpool.tile(
                            [128, 2], F32, tag=f"mv{kt}", name=f"mv{s}_{kt}"
                        )
                        nc.vector.bn_aggr(out=mv, in_=stats)
                        # [E[x], E[x^2]] = [mean, var + mean^2], cast to f32r
                        nc.vector.tensor_mul(
                            tmp_all[:, kt : kt + 1], mv[:, 0:1], mv[:, 0:1]
                        )
                        nc.vector.tensor_add(
                            s2_all[:, kt, 1:2], tmp_all[:, kt : kt + 1], mv[:, 1:2]
                        )
                        nc.vector.tensor_copy(s2_all[:, kt, 0:1], mv[:, 0:1])
                # one matmul folds the 16-partition groups for all kt at once
                ps_gs = ps.tile([8, KT, 2], F32, tag="mm", name=f"ps_gs{s}")
                nc.tensor.matmul(ps_gs, ind_sb, s2_all, start=True, stop=True)
                grst = gnpool.tile([8, KT, 2], F32, tag="grst", name=f"grst{s}")
                msq = gnpool.tile([8, KT], F32, tag="msq", name=f"msq{s}")
                nc.vector.tensor_scalar_mul(grst, ps_gs, 1.0 / GS)
                nc.vector.tensor_mul(msq, grst[:, :, 0], grst[:, :, 0])
                nc.vector.tensor_sub(grst[:, :, 1], grst[:, :, 1], msq)
                nc.scalar.activation(
                    out=grst[:, :, 1], in_=grst[:, :, 1], func=Sqrt, bias=eps_sb
                )
                nc.vector.reciprocal(grst[:, :, 1], grst[:, :, 1])
                mr = gnpool.tile([8, KT, 2], F32R, tag="mr", name=f"mr{s}")
                nc.vector.tensor_copy(mr, grst)
                return mr

            def gn_apply(s, mr, engines):
                """Broadcast stats to channels and apply x*scale+bias -> h bf16.

                One broadcast matmul + 3 strided DVE ops for all kt."""
                x_sb = xbf_tiles[s]
                h_sb = hpool.tile([128, KT, N], BF16, tag="h", name=f"h{s}")
                ps_bc = ps.tile([128, KT, 2], F32, tag="mm", name=f"ps_bc{s}")
                nc.tensor.matmul(ps_bc, indT_sb, mr, start=True, stop=True)
                scb = gnpool.tile([128, KT, 2], F32, tag="scb", name=f"scb{s}")
                nc.vector.tensor_mul(scb[:, :, 0], ps_bc[:, :, 1], nw_sb)
                nc.vector.tensor_mul(scb[:, :, 1], ps_bc[:, :, 0], scb[:, :, 0])
                nc.vector.tensor_sub(scb[:, :, 1], nb_sb, scb[:, :, 1])
                for kt in range(KT):
                    if engines[kt] in ("v", "g"):
                        eng = nc.vector if engines[kt] == "v" else nc.gpsimd
                        eng.tensor_scalar(
                            out=h_sb[:, kt, :],
                            in0=x_sb[:, kt, :],
                            scalar1=scb[:, kt, 0:1],
                            scalar2=scb[:, kt, 1:2],
                            op0=mult,
                            op1=add,
                        )
                    else:
                        nc.scalar.activation(
                            out=h_sb[:, kt, :], in_=x_sb[:, kt, :],
                            func=Identity, scale=scb[:, kt, 0:1],
                            bias=scb[:, kt, 1:2],
                        )
                return h_sb

            def v_gemm(s, h_sb):
                """vT = h^T @ v_w^T (no bias — folded into proj bias)."""
                vT_sb = vpool.tile([128, NT, C], BF16, tag="vT", name=f"vT{s}")
                for nt in range(NT):
                    pm = ps.tile([128, 512], F32, tag="mm")
                    for kt in range(KT):
                        nc.tensor.matmul(
                            pm,
                            h_sb[:, kt, nt * 128 : (nt + 1) * 128],
                            wvT_sb[:, kt, :],
                            start=(kt == 0),
                            stop=(kt == KT - 1),
                        )
                    nc.vector.tensor_copy(vT_sb[:, nt, :], pm)
                return vT_sb

            def t_gemm(s, h_sb):
                """t = M2 @ h + r (bias folds the q-bias key-row term)."""
                t_sb = tpool.tile([128, KT, N], BF16, tag="t", name=f"t{s}")
                for ih in range(NH):
                    for ct in range(KT):
                        pm = ps.tile([128, 512], F32, tag="mm")
                        for kt in range(KT):
                            nc.tensor.matmul(
                                pm,
                                m2T_sb[:, kt, ct * 128 : (ct + 1) * 128],
                                h_sb[:, kt, ih * 512 : (ih + 1) * 512],
                                start=(kt == 0),
                                stop=(kt == KT - 1),
                            )
                        nc.scalar.activation(
                            out=t_sb[:, ct, ih * 512 : (ih + 1) * 512],
                            in_=pm,
                            func=Identity,
                            bias=rcol_sb[:, ct : ct + 1],
                        )
                return t_sb

            def attn_scores(s, h_sb, t_sb):
                # s = h^T t (keys j on partitions); e = exp(s * scale)
                e_sb = epool.tile([128, NT, N], BF16, tag="e", name=f"e{s}")
                for jt in range(NT):
                    for ih in range(NH):
                        pm = ps.tile([128, 512], F32, tag="mm")
                        for kt in range(KT):
                            nc.tensor.matmul(
                                pm,
                                h_sb[:, kt, jt * 128 : (jt + 1) * 128],
                                t_sb[:, kt, ih * 512 : (ih + 1) * 512],
                                start=(kt == 0),
                                stop=(kt == KT - 1),
                            )
                        nc.scalar.activation(
                            out=e_sb[:, jt, ih * 512 : (ih + 1) * 512],
                            in_=pm,
                            func=Exp,
                            scale=SCALE,
                        )
                return e_sb

            def attn_out(s, e_sb, vT_sb):
                # softmax denominators S = sum_j e; 1/S broadcast via DRAM
                recipS = spool.tile([1, N], F32, tag="recipS", name=f"recipS{s}")
                for ih in range(NH):
                    pS = ps.tile([1, 512], F32, tag="mm")
                    for jt in range(NT):
                        nc.tensor.matmul(
                            pS,
                            ones_col,
                            e_sb[:, jt, ih * 512 : (ih + 1) * 512],
                            start=(jt == 0),
                            stop=(jt == NT - 1),
                        )
                    nc.vector.reciprocal_approx_fast(
                        out=recipS[:, ih * 512 : (ih + 1) * 512], in_=pS
                    )
                nc.sync.dma_start(out=sdram.ap()[s].unsqueeze(0), in_=recipS)
                rSbc = spool.tile([128, N], F32, tag="rSbc", name=f"rSbc{s}")
                nc.sync.dma_start(
                    out=rSbc, in_=sdram.ap()[s].partition_broadcast(128)
                )
                # o = vT^T @ e, normalized by 1/S
                o_sb = opool.tile([128, KT, N], BF16, tag="o", name=f"o{s}")
                for ct in range(KT):
                    for ih in range(NH):
                        pm = ps.tile([128, 512], F32, tag="mm")
                        for jt in range(NT):
                            nc.tensor.matmul(
                                pm,
                                vT_sb[:, jt, ct * 128 : (ct + 1) * 128],
                                e_sb[:, jt, ih * 512 : (ih + 1) * 512],
                                start=(jt == 0),
                                stop=(jt == NT - 1),
                            )
                        nc.vector.tensor_mul(
                            o_sb[:, ct, ih * 512 : (ih + 1) * 512],
                            pm,
                            rSbc[:, ih * 512 : (ih + 1) * 512],
                        )
                return o_sb

            def proj_resid(s, o_sb, x_sb):
                # residual accumulates in place into the (now dead) x tile
                for ct2 in range(KT):
                    for ih in range(NH):
                        pm = ps.tile([128, 512], F32, tag="mm")
                        for ckt in range(KT):
                            nc.tensor.matmul(
                                pm,
                                projw_sb[:, ckt, ct2 * 128 : (ct2 + 1) * 128],
                                o_sb[:, ckt, ih * 512 : (ih + 1) * 512],
                                start=(ckt == 0),
                                stop=(ckt == KT - 1),
                            )
                        # + proj bias (v-bias folded in), in place on PSUM
                        nc.scalar.activation(
                            out=pm, in_=pm, func=Identity,
                            bias=pb_col[:, ct2 : ct2 + 1],
                        )
                        # + residual, in place into x
                        nc.vector.tensor_add(
                            x_sb[:, ct2, ih * 512 : (ih + 1) * 512],
                            pm,
                            x_sb[:, ct2, ih * 512 : (ih + 1) * 512],
                        )
                        nc.gpsimd.dma_start(
                            out=y_ext.ap()[
                                s,
                                ct2 * 128 : (ct2 + 1) * 128,
                                ih * 512 : (ih + 1) * 512,
                            ],
                            in_=x_sb[:, ct2, ih * 512 : (ih + 1) * 512],
                        )

            # ---- interleaved two-sample schedule ----
            mr0 = gn_stats(0)
            h0 = gn_apply(0, mr0, engines="gaag")
            pe_warm(WARMUP_MM2, 100)
            vT0 = v_gemm(0, h0)
            t0 = t_gemm(0, h0)
            mr1 = gn_stats(1)  # DVE-only, hides under s0 attn
            e0 = attn_scores(0, h0, t0)
            h1 = gn_apply(1, mr1, engines="gaag")  # keep DVE free for s0 attn
            o0 = attn_out(0, e0, vT0)
            proj_resid(0, o0, x_tiles[0])
            vT1 = v_gemm(1, h1)
            t1 = t_gemm(1, h1)
            e1 = attn_scores(1, h1, t1)
            o1 = attn_out(1, e1, vT1)
            proj_resid(1, o1, x_tiles[1])

    nc.compile()
    return nc


def _get_nc():
    if "nc" not in _BUILD_CACHE:
        _BUILD_CACHE["nc"] = _build()
    return _BUILD_CACHE["nc"]


def kernel(x, norm_w, norm_b, qkv_w, qkv_b, proj_w, proj_b, _trace=False):
    global LAST_RESULT
    nc = _get_nc()

    x = np.asarray(x, dtype=np.float32).reshape(B, C, N)
    qkv_w = np.asarray(qkv_w, dtype=np.float32)
    qkv_b = np.asarray(qkv_b, dtype=np.float32)
    norm_w = np.asarray(norm_w, dtype=np.float32)
    norm_b = np.asarray(norm_b, dtype=np.float32)
    proj_w = np.asarray(proj_w, dtype=np.float32)
    proj_b = np.asarray(proj_b, dtype=np.float32)

    Wq, Wk, Wv = qkv_w[:C], qkv_w[C : 2 * C], qkv_w[2 * C :]
    # m2T[d, c] = M2[c, d] with M2 = Wk^T Wq, so m2T = Wq^T Wk
    m2T = (Wq.T.astype(np.float64) @ Wk.astype(np.float64)).astype(NPBF)
    wvT = np.ascontiguousarray(Wv.T).astype(NPBF)
    projwT = np.ascontiguousarray(proj_w.T).astype(NPBF)
    # r folds the q-bias key-row term of the scores; pb2 folds the v-bias
    r = (Wk.T @ qkv_b[:C]).astype(np.float32)
    pb2 = (proj_b + proj_w @ qkv_b[2 * C :]).astype(np.float32)
    ind16 = np.zeros((128, 8), dtype=np.float32)
    for p in range(128):
        ind16[p, p // GS] = 1.0
    ind16T = np.ascontiguousarray(ind16.T)

    consts12 = np.ascontiguousarray(
        np.concatenate(
            [
                norm_w.reshape(KT, 128).T,
                norm_b.reshape(KT, 128).T,
                pb2.reshape(KT, 128).T,
            ],
            axis=1,
        )
    )
    r_col = np.ascontiguousarray(r.reshape(KT, 128).T)
    shared = {
        "m2T": np.ascontiguousarray(m2T),
        "wvT": wvT,
        "projwT": projwT,
        "consts12": consts12,
        "r_col": r_col,
        "ind16": ind16,
        "ind16T": ind16T,
        "onesb": np.ones(128, dtype=NPBF),
    }
    xbf = x.astype(NPBF)
    in_maps = [
        {
            "x": np.ascontiguousarray(x[c * SPC : (c + 1) * SPC]),
            "xbf": np.ascontiguousarray(xbf[c * SPC : (c + 1) * SPC]),
            **shared,
        }
        for c in range(NCORES)
    ]
    res = run_bass_kernel_spmd(nc, in_maps, list(range(NCORES)), trace=_trace)
    LAST_RESULT = res
    out = np.concatenate([res.results[i]["y"] for i in range(NCORES)], axis=0)
    return out.reshape(B, C, H, W)


# revision 13
# speedup vs baseline: 1.1051x; 1.1051x over previous
"""AttnBlock (GroupNorm + 1x1-conv QKV + single-head spatial attention + proj
+ residual) on 8 Trainium2 NeuronCores.

Sharding: pure data-parallel over batch — 16 samples / 8 cores = 2 samples per
core; weights broadcast. No collectives; gather on host.

Per-core formulation (per sample, C=512 channels, N=1024 spatial), all big
GEMMs in bf16 (fp32 PSUM accumulation; rel-err ~3e-3 vs fp32 reference):
  h   = groupnorm(x)                   (stats via PE indicator matmuls; h bf16)
  vT  = h^T @ v_w^T                    (spatial on partitions, C free)
  t   = M2 @ h + r                     (M2 = Wk^T Wq host-precomputed;
                                        r = Wk^T q_b folds the q-bias row term)
  s   = h^T t                          (== k^T q up to softmax-invariant terms)
  e   = exp(s * C^-0.5)                (logits O(1); no max-subtraction)
  S   = ones^T e                       (softmax denominators via PE reduction)
  o   = vT^T e * (1/S)                 (1/S broadcast across partitions via DRAM)
  y   = x + proj_w @ o + pb2           (pb2 = proj_b + proj_w @ v_b folds v-bias)
The M2 trick removes one C x C x N GEMM per sample and the q/k bias passes;
dropped scores terms are constant over keys and cancel in softmax — exact.
bf16 stationary operands keep LDWEIGHTS (~107ns) hidden under the 213ns
matmul stream, vs fp32r's 224ns two-pass load that gated the fp32 baseline.
A burst of dummy matmuls during the DMA/GroupNorm head keeps the PE HAM
un-throttled (K=8/8, 2.4GHz) when the real GEMMs arrive.
"""

import numpy as np
import ml_dtypes

import concourse.bass as bass
import concourse.tile as tile
from concourse import bacc, mybir
from concourse.bass_utils import run_bass_kernel_spmd

B, C, H, W = 16, 512, 32, 32
N = H * W              # 1024 spatial positions
G = 32                 # groups
GS = C // G            # 16 channels per group
NCORES = 8
SPC = B // NCORES      # samples per core
EPS = 1e-6
SCALE = float(C) ** -0.5
KT = C // 128          # 4 channel tiles of 128
NT = N // 128          # 8 spatial tiles of 128
NH = N // 512          # 2 free-dim halves of 512

F32 = mybir.dt.float32
F32R = mybir.dt.float32r
BF16 = mybir.dt.bfloat16
NPBF = ml_dtypes.bfloat16

WARMUP_MM = 24         # dummy PE matmuls during the head to keep HAM warm
WARMUP_MM2 = 6         # second warmup burst while the h-applies run

_BUILD_CACHE = {}
LAST_RESULT = None  # BassKernelResults of the most recent run (for test harness)


def _build():
    nc = bacc.Bacc("TRN2", target_bir_lowering=False, debug=False)

    x_ext = nc.declare_dram_parameter("x", [SPC, C, N], F32, isOutput=False)
    xbf_ext = nc.declare_dram_parameter("xbf", [SPC, C, N], BF16, isOutput=False)
    m2T_ext = nc.declare_dram_parameter("m2T", [C, C], BF16, isOutput=False)
    wvT_ext = nc.declare_dram_parameter("wvT", [C, C], BF16, isOutput=False)
    projwT_ext = nc.declare_dram_parameter("projwT", [C, C], BF16, isOutput=False)
    cst_ext = nc.declare_dram_parameter("consts12", [128, 12], F32, isOutput=False)
    rcol_ext = nc.declare_dram_parameter("r_col", [128, KT], F32, isOutput=False)
    ind_ext = nc.declare_dram_parameter("ind16", [128, 8], F32R, isOutput=False)
    indT_ext = nc.declare_dram_parameter("ind16T", [8, 128], F32R, isOutput=False)
    ones_ext = nc.declare_dram_parameter("onesb", [128], BF16, isOutput=False)
    y_ext = nc.declare_dram_parameter("y", [SPC, C, N], F32, isOutput=True)

    sdram = nc.dram_tensor("rs_bounce", [SPC, N], F32)

    Identity = mybir.ActivationFunctionType.Identity
    Exp = mybir.ActivationFunctionType.Exp
    Sqrt = mybir.ActivationFunctionType.Sqrt
    Square = mybir.ActivationFunctionType.Square
    mult = mybir.AluOpType.mult
    add = mybir.AluOpType.add

    with tile.TileContext(nc) as tc:
        with (
            tc.tile_pool(name="wpool", bufs=1) as wpool,
            tc.tile_pool(name="cpool", bufs=1) as cpool,
            tc.tile_pool(name="xpool", bufs=2) as xpool,
            tc.tile_pool(name="hpool", bufs=2) as hpool,
            tc.tile_pool(name="tpool", bufs=2) as tpool,
            tc.tile_pool(name="vpool", bufs=2) as vpool,
            tc.tile_pool(name="epool", bufs=2) as epool,
            tc.tile_pool(name="opool", bufs=2) as opool,
            tc.tile_pool(name="gnpool", bufs=2) as gnpool,
            tc.tile_pool(name="spool", bufs=2) as spool,
            tc.tile_pool(name="ps", bufs=8, space="PSUM") as ps,
        ):
            # ---- x sample 0: one DMA per kt, spread across idle queues so
            # the issue slots don't serialize and kt0 lands earliest ----
            x_tiles = []
            for s in range(SPC):
                x_tiles.append(
                    xpool.tile([128, KT, N], F32, tag="x", name=f"x_sb{s}")
                )
            # bf16 shadow of x feeds GroupNorm stats + apply: half the HBM
            # bytes of the fp32 x, so the head is off the DMA floor. The fp32
            # x (residual-only) streams in later during the quiet window.
            xbf_tiles = []
            for s in range(SPC):
                xbf_tiles.append(
                    xpool.tile([128, KT, N], BF16, tag="xbf", name=f"xbf_sb{s}")
                )
            # sg0 halves on the sync ring, sg1 halves on the gpsimd ring:
            # two rings stream in parallel and kt_i is complete early.
            for kt in range(KT):
                nc.sync.dma_start(
                    out=xbf_tiles[0][:, kt, 0:512],
                    in_=xbf_ext.ap()[0, kt * 128 : (kt + 1) * 128, 0:512],
                )
            for kt in range(KT):
                nc.gpsimd.dma_start(
                    out=xbf_tiles[0][:, kt, 512:1024],
                    in_=xbf_ext.ap()[0, kt * 128 : (kt + 1) * 128, 512:1024],
                )

            # ---- small constants (sync ring, right behind x0-sg0) ----
            cst_sb = cpool.tile([128, 12], F32)
            nc.sync.dma_start(out=cst_sb, in_=cst_ext.ap())
            nw_sb = cst_sb[:, 0:4]
            nb_sb = cst_sb[:, 4:8]
            pb_col = cst_sb[:, 8:12]
            ind_sb = cpool.tile([128, 8], F32R)
            nc.sync.dma_start(out=ind_sb, in_=ind_ext.ap())
            indT_sb = cpool.tile([8, 128], F32R)
            nc.sync.dma_start(out=indT_sb, in_=indT_ext.ap())
            eps_sb = cpool.tile([8, 1], F32)
            nc.vector.memset(eps_sb, EPS)
            # ACT table warmup: Sqrt-set then Exp-set (Identity/Square are in
            # every set, so exactly these two table loads happen)
            warm_sb = cpool.tile([8, 1], F32)
            nc.scalar.activation(out=warm_sb, in_=eps_sb, func=Sqrt)
            nc.scalar.activation(out=warm_sb, in_=eps_sb, func=Exp)

            # ---- PE warmup: keep HAM at K=8/8 through the head ----
            wmA = cpool.tile([128, 128], BF16, name="wmA")
            wmB = cpool.tile([128, 512], BF16, name="wmB")
            nc.vector.memset(wmA, 0.5)
            nc.vector.memset(wmB, 0.5)

            def pe_warm(n, base):
                for i in range(n):
                    pwt = ps.tile([128, 512], F32, tag="mm", name=f"warm{base+i}")
                    nc.tensor.matmul(pwt, wmA, wmB, start=True, stop=True)

            pe_warm(WARMUP_MM, 0)

            # ---- weights: wvT behind x0-sg1 on gpsimd (first big consumer);
            # m2T + the rest behind the consts on sync ----
            wvT_sb = wpool.tile([128, KT, C], BF16, name="wvT")
            m2T_sb = wpool.tile([128, KT, C], BF16, name="m2T")
            projw_sb = wpool.tile([128, KT, C], BF16, name="projw")
            nc.gpsimd.dma_start(
                out=wvT_sb, in_=wvT_ext.ap().rearrange("(k p) c -> p k c", p=128)
            )
            # sync ring, strictly ordered: xbf1 streams BEFORE the fat fp32
            # x loads; the fp32 x + projw go on the gpsimd ring (ahead of the
            # y stores) so the sync ring stays free for the 1/S bounce DMAs.
            nc.sync.dma_start(
                out=xbf_tiles[1],
                in_=xbf_ext.ap()[1].rearrange("(k p) n -> p k n", p=128),
            )
            for s in range(SPC):
                nc.gpsimd.dma_start(
                    out=x_tiles[s],
                    in_=x_ext.ap()[s].rearrange("(k p) n -> p k n", p=128),
                )
            nc.gpsimd.dma_start(
                out=projw_sb,
                in_=projwT_ext.ap().rearrange("(k p) c -> p k c", p=128),
            )
            rcol_sb = cpool.tile([128, KT], F32)
            nc.sync.dma_start(out=rcol_sb, in_=rcol_ext.ap())
            ones_col = cpool.tile([128, 1], BF16)
            nc.sync.dma_start(out=ones_col, in_=ones_ext.ap().unsqueeze(1))
            nc.sync.dma_start(
                out=m2T_sb, in_=m2T_ext.ap().rearrange("(k p) c -> p k c", p=128)
            )

            def gn_stats(s, act_kts=()):
                """GroupNorm stats for sample s -> mr (8,KT,2) F32R [mean,rstd].

                Batched small-op chain: one indicator matmul for all kt.
                act_kts: kt indices whose moments run on ScalarE (Identity /
                Square with accum_out) so DVE and ACT work concurrently.
                """
                x_sb = xbf_tiles[s]
                s2_all = gnpool.tile([128, KT, 2], F32R, tag="s2", name=f"s2_{s}")
                tmp_all = gnpool.tile([128, KT], F32, tag="s2t", name=f"s2t{s}")
                for kt in range(KT):
                    if kt in act_kts:
                        s2f = gnpool.tile(
                            [128, 2], F32, tag=f"s2f{kt}", name=f"s2f{s}_{kt}"
                        )
                        scr = gnpool.tile(
                            [128, N], F32, tag="gnscr", name=f"scr{s}_{kt}"
                        )
                        nc.scalar.activation(
                            out=scr, in_=x_sb[:, kt, :], func=Identity,
                            scale=1.0 / N, accum_out=s2f[:, 0:1],
                        )
                        scr2 = gnpool.tile(
                            [128, N], F32, tag="gnscr", name=f"scr2{s}_{kt}"
                        )
                        nc.scalar.activation(
                            out=scr2, in_=x_sb[:, kt, :], func=Square,
                            scale=N ** -0.5, accum_out=s2f[:, 1:2],
                        )
                        nc.vector.tensor_copy(s2_all[:, kt, :], s2f)
                    else:
                        stats = gnpool.tile(
                            [128, 2, 6], F32, tag=f"stats{kt}", name=f"stats{s}_{kt}"
                        )
                        for sg in range(2):
                            nc.vector.bn_stats(
                                out=stats[:, sg, :],
                                in_=x_sb[:, kt, sg * 512 : (sg + 1) * 512],
                            )
                        mv = gnpool.tile(
                            [128, 2], F32, tag=f"mv{kt}", name=f"mv{s}_{kt}"
                        )
                        nc.vector.bn_aggr(out=mv, in_=stats)
                        # [E[x], E[x^2]] = [mean, var + mean^2], cast to f32r
                        nc.vector.tensor_mul(
                            tmp_all[:, kt : kt + 1], mv[:, 0:1], mv[:, 0:1]
                        )
                        nc.vector.tensor_add(
                            s2_all[:, kt, 1:2], tmp_all[:, kt : kt + 1], mv[:, 1:2]
                        )
                        nc.vector.tensor_copy(s2_all[:, kt, 0:1], mv[:, 0:1])
                # one matmul folds the 16-partition groups for all kt at once
                ps_gs = ps.tile([8, KT, 2], F32, tag="mm", name=f"ps_gs{s}")
                nc.tensor.matmul(ps_gs, ind_sb, s2_all, start=True, stop=True)
                grst = gnpool.tile([8, KT, 2], F32, tag="grst", name=f"grst{s}")
                msq = gnpool.tile([8, KT], F32, tag="msq", name=f"msq{s}")
                nc.vector.tensor_scalar_mul(grst, ps_gs, 1.0 / GS)
                nc.vector.tensor_mul(msq, grst[:, :, 0], grst[:, :, 0])
                nc.vector.tensor_sub(grst[:, :, 1], grst[:, :, 1], msq)
                nc.scalar.activation(
                    out=grst[:, :, 1], in_=grst[:, :, 1], func=Sqrt, bias=eps_sb
                )
                nc.vector.reciprocal(grst[:, :, 1], grst[:, :, 1])
                mr = gnpool.tile([8, KT, 2], F32R, tag="mr", name=f"mr{s}")
                nc.vector.tensor_copy(mr, grst)
                return mr

            def gn_apply(s, mr, engines):
                """Broadcast stats to channels and apply x*scale+bias -> h bf16.

                One broadcast matmul + 3 strided DVE ops for all kt."""
                x_sb = xbf_tiles[s]
                h_sb = hpool.tile([128, KT, N], BF16, tag="h", name=f"h{s}")
                ps_bc = ps.tile([128, KT, 2], F32, tag="mm", name=f"ps_bc{s}")
                nc.tensor.matmul(ps_bc, indT_sb, mr, start=True, stop=True)
                scb = gnpool.tile([128, KT, 2], F32, tag="scb", name=f"scb{s}")
                nc.vector.tensor_mul(scb[:, :, 0], ps_bc[:, :, 1], nw_sb)
                nc.vector.tensor_mul(scb[:, :, 1], ps_bc[:, :, 0], scb[:, :, 0])
                nc.vector.tensor_sub(scb[:, :, 1], nb_sb, scb[:, :, 1])
                for kt in range(KT):
                    if engines[kt] in ("v", "g"):
                        eng = nc.vector if engines[kt] == "v" else nc.gpsimd
                        eng.tensor_scalar(
                            out=h_sb[:, kt, :],
                            in0=x_sb[:, kt, :],
                            scalar1=scb[:, kt, 0:1],
                            scalar2=scb[:, kt, 1:2],
                            op0=mult,
                            op1=add,
                        )
                    else:
                        nc.scalar.activation(
                            out=h_sb[:, kt, :], in_=x_sb[:, kt, :],
                            func=Identity, scale=scb[:, kt, 0:1],
                            bias=scb[:, kt, 1:2],
                        )
                return h_sb

            def v_gemm(s, h_sb):
                """vT = h^T @ v_w^T (no bias — folded into proj bias)."""
                vT_sb = vpool.tile([128, NT, C], BF16, tag="vT", name=f"vT{s}")
                for nt in range(NT):
                    pm = ps.tile([128, 512], F32, tag="mm")
                    for kt in range(KT):
                        nc.tensor.matmul(
                            pm,
                            h_sb[:, kt, nt * 128 : (nt + 1) * 128],
                            wvT_sb[:, kt, :],
                            start=(kt == 0),
                            stop=(kt == KT - 1),
                        )
                    nc.vector.tensor_copy(vT_sb[:, nt, :], pm)
                return vT_sb

            def t_gemm(s, h_sb):
                """t = M2 @ h + r (bias folds the q-bias key-row term)."""
                t_sb = tpool.tile([128, KT, N], BF16, tag="t", name=f"t{s}")
                for ih in range(NH):
                    for ct in range(KT):
                        pm = ps.tile([128, 512], F32, tag="mm")
                        for kt in range(KT):
                            nc.tensor.matmul(
                                pm,
                                m2T_sb[:, kt, ct * 128 : (ct + 1) * 128],
                                h_sb[:, kt, ih * 512 : (ih + 1) * 512],
                                start=(kt == 0),
                                stop=(kt == KT - 1),
                            )
                        nc.scalar.activation(
                            out=t_sb[:, ct, ih * 512 : (ih + 1) * 512],
                            in_=pm,
                            func=Identity,
                            bias=rcol_sb[:, ct : ct + 1],
                        )
                return t_sb

            def attn_scores(s, h_sb, t_sb):
                # s = h^T t (keys j on partitions); e = exp(s * scale)
                e_sb = epool.tile([128, NT, N], BF16, tag="e", name=f"e{s}")
                for jt in range(NT):
                    for ih in range(NH):
                        pm = ps.tile([128, 512], F32, tag="mm")
                        for kt in range(KT):
                            nc.tensor.matmul(
                                pm,
                                h_sb[:, kt, jt * 128 : (jt + 1) * 128],
                                t_sb[:, kt, ih * 512 : (ih + 1) * 512],
                                start=(kt == 0),
                                stop=(kt == KT - 1),
                            )
                        nc.scalar.activation(
                            out=e_sb[:, jt, ih * 512 : (ih + 1) * 512],
                            in_=pm,
                            func=Exp,
                            scale=SCALE,
                        )
                return e_sb

            def attn_out(s, e_sb, vT_sb):
                # softmax denominators S = sum_j e; 1/S broadcast via DRAM
                recipS = spool.tile([1, N], F32, tag="recipS", name=f"recipS{s}")
                for ih in range(NH):
                    pS = ps.tile([1, 512], F32, tag="mm")
                    for jt in range(NT):
                        nc.tensor.matmul(
                            pS,
                            ones_col,
                            e_sb[:, jt, ih * 512 : (ih + 1) * 512],
                            start=(jt == 0),
                            stop=(jt == NT - 1),
                        )
                    nc.vector.reciprocal_approx_fast(
                        out=recipS[:, ih * 512 : (ih + 1) * 512], in_=pS
                    )
                nc.sync.dma_start(out=sdram.ap()[s].unsqueeze(0), in_=recipS)
                rSbc = spool.tile([128, N], F32, tag="rSbc", name=f"rSbc{s}")
                nc.sync.dma_start(
                    out=rSbc, in_=sdram.ap()[s].partition_broadcast(128)
                )
                # o = vT^T @ e, normalized by 1/S
                o_sb = opool.tile([128, KT, N], BF16, tag="o", name=f"o{s}")
                for ct in range(KT):
                    for ih in range(NH):
                        pm = ps.tile([128, 512], F32, tag="mm")
                        for jt in range(NT):
                            nc.tensor.matmul(
                                pm,
                                vT_sb[:, jt, ct * 128 : (ct + 1) * 128],
                                e_sb[:, jt, ih * 512 : (ih + 1) * 512],
                                start=(jt == 0),
                                stop=(jt == NT - 1),
                            )
                        nc.vector.tensor_mul(
                            o_sb[:, ct, ih * 512 : (ih + 1) * 512],
                            pm,
                            rSbc[:, ih * 512 : (ih + 1) * 512],
                        )
                return o_sb

            def proj_resid(s, o_sb, x_sb):
                # residual accumulates in place into the (now dead) x tile
                for ct2 in range(KT):
                    for ih in range(NH):
                        pm = ps.tile([128, 512], F32, tag="mm")
                        for ckt in range(KT):
                            nc.tensor.matmul(
                                pm,
                                projw_sb[:, ckt, ct2 * 128 : (ct2 + 1) * 128],
                                o_sb[:, ckt, ih * 512 : (ih + 1) * 512],
                                start=(ckt == 0),
                                stop=(ckt == KT - 1),
                            )
                        # + proj bias (v-bias folded in), in place on PSUM
                        nc.scalar.activation(
                            out=pm, in_=pm, func=Identity,
                            bias=pb_col[:, ct2 : ct2 + 1],
                        )
                        # + residual, in place into x
                        nc.vector.tensor_add(
                            x_sb[:, ct2, ih * 512 : (ih + 1) * 512],
                            pm,
                            x_sb[:, ct2, ih * 512 : (ih + 1) * 512],
                        )
                        nc.gpsimd.dma_start(
                            out=y_ext.ap()[
                                s,
                                ct2 * 128 : (ct2 + 1) * 128,
                                ih * 512 : (ih + 1) * 512,
                            ],
                            in_=x_sb[:, ct2, ih * 512 : (ih + 1) * 512],
                        )

            # ---- interleaved two-sample schedule ----
            mr0 = gn_stats(0)
            h0 = gn_apply(0, mr0, engines="gaag")
            pe_warm(WARMUP_MM2, 100)
            vT0 = v_gemm(0, h0)
            t0 = t_gemm(0, h0)
            mr1 = gn_stats(1)  # DVE-only, hides under s0 attn
            e0 = attn_scores(0, h0, t0)
            h1 = gn_apply(1, mr1, engines="gaag")  # keep DVE free for s0 attn
            o0 = attn_out(0, e0, vT0)
            proj_resid(0, o0, x_tiles[0])
            vT1 = v_gemm(1, h1)
            t1 = t_gemm(1, h1)
            e1 = attn_scores(1, h1, t1)
            o1 = attn_out(1, e1, vT1)
            proj_resid(1, o1, x_tiles[1])

    nc.compile()
    return nc


def _get_nc():
    if "nc" not in _BUILD_CACHE:
        _BUILD_CACHE["nc"] = _build()
    return _BUILD_CACHE["nc"]


def kernel(x, norm_w, norm_b, qkv_w, qkv_b, proj_w, proj_b, _trace=False):
    global LAST_RESULT
    nc = _get_nc()

    x = np.asarray(x, dtype=np.float32).reshape(B, C, N)
    qkv_w = np.asarray(qkv_w, dtype=np.float32)
    qkv_b = np.asarray(qkv_b, dtype=np.float32)
    norm_w = np.asarray(norm_w, dtype=np.float32)
    norm_b = np.asarray(norm_b, dtype=np.float32)
    proj_w = np.asarray(proj_w, dtype=np.float32)
    proj_b = np.asarray(proj_b, dtype=np.float32)

    Wq, Wk, Wv = qkv_w[:C], qkv_w[C : 2 * C], qkv_w[2 * C :]
    # m2T[d, c] = M2[c, d] with M2 = Wk^T Wq, so m2T = Wq^T Wk
    m2T = (Wq.T.astype(np.float64) @ Wk.astype(np.float64)).astype(NPBF)
    wvT = np.ascontiguousarray(Wv.T).astype(NPBF)
    projwT = np.ascontiguousarray(proj_w.T).astype(NPBF)
    # r folds the q-bias key-row term of the scores; pb2 folds the v-bias
    r = (Wk.T @ qkv_b[:C]).astype(np.float32)
    pb2 = (proj_b + proj_w @ qkv_b[2 * C :]).astype(np.float32)
    ind16 = np.zeros((128, 8), dtype=np.float32)
    for p in range(128):
        ind16[p, p // GS] = 1.0
    ind16T = np.ascontiguousarray(ind16.T)

    consts12 = np.ascontiguousarray(
        np.concatenate(
            [
                norm_w.reshape(KT, 128).T,
                norm_b.reshape(KT, 128).T,
                pb2.reshape(KT, 128).T,
            ],
            axis=1,
        )
    )
    r_col = np.ascontiguousarray(r.reshape(KT, 128).T)
    shared = {
        "m2T": np.ascontiguousarray(m2T),
        "wvT": wvT,
        "projwT": projwT,
        "consts12": consts12,
        "r_col": r_col,
        "ind16": ind16,
        "ind16T": ind16T,
        "onesb": np.ones(128, dtype=NPBF),
    }
    xbf = x.astype(NPBF)
    in_maps = [
        {
            "x": np.ascontiguousarray(x[c * SPC : (c + 1) * SPC]),
            "xbf": np.ascontiguousarray(xbf[c * SPC : (c + 1) * SPC]),
            **shared,
        }
        for c in range(NCORES)
    ]
    res = run_bass_kernel_spmd(nc, in_maps, list(range(NCORES)), trace=_trace)
    LAST_RESULT = res
    out = np.concatenate([res.results[i]["y"] for i in range(NCORES)], axis=0)
    return out.reshape(B, C, H, W)
